# revision 1
# baseline (speedup 1.0000x reference)
"""Trainium2 Bass kernel for nn_CrossAttentionBlock (B=8, C=256, H=W=48).

Sharding: data-parallel over batch B — one batch per NeuronCore (8 cores).

Per-core math (x: [C=256, N=2304] f32):
  LayerNorm over C is folded:
    - w_n / b_n folded into projection weights on host:
        W_eff = W * w_n[None,:],  b_eff = b + W @ b_n
    - attention SCALE folded into Wq_eff / bq_eff
    - per-location mean u[n] / rstd[n] computed on-device via a
      ones-matrix matmul (broadcasts u across all 128 partitions in one
      matmul), then xn = (x - u_b) * rstd_b on DVE.
  Attention is computed transposed:  St[m,n] = sum_o k[o,m] q[o,n]
  so softmax normalization runs over the *partition* axis m:
    - no row-max subtraction (logits bounded ~21, exp safe in f32)
    - P = exp(St) (ScalarE, PSUM->SBUF bf16 eviction)
    - rowsum[n] = sum_m P[m,n] via an M=1 ones-matmul, folded out as
      1/rowsum AFTER the output projection (scaling commutes with Wp).
  v is produced directly transposed (vT[m,o] = sum_c xn2[c,m] WvT[c,o])
  so P·V contracts over m on partitions with zero PE transposes.
"""

import os
import sys
import types
import ctypes
import contextlib

sys.path.insert(0, "/opt/trn_rl_repo")

import numpy as np
import ml_dtypes

# ---------------------------------------------------------------------------
# NTFF profile hook stub (antenv.axon_hooks is absent in this container; the
# ctypes shim mirrors trn_agent_boot). Only used when tracing is requested.
# ---------------------------------------------------------------------------


def _ntff_profile_via_ctypes(so_path):
    try:
        lib = ctypes.CDLL(so_path)
    except OSError:
        return None
    if not hasattr(lib, "axon_start_nrt_profile"):
        return None
    lib.axon_start_nrt_profile.argtypes = [
        ctypes.POINTER(ctypes.c_int64),
        ctypes.c_size_t,
    ]
    lib.axon_start_nrt_profile.restype = ctypes.c_int64
    lib.axon_stop_nrt_profile.argtypes = [ctypes.c_char_p]
    lib.axon_stop_nrt_profile.restype = ctypes.c_int64

    @contextlib.contextmanager
    def _hook(output_dir, device_ids):
        import jax

        jax.devices()
        if device_ids:
            ids = (ctypes.c_int64 * len(device_ids))(*device_ids)
            rc = lib.axon_start_nrt_profile(ids, len(device_ids))
        else:
            rc = lib.axon_start_nrt_profile(None, 0)
        if rc != 0:
            raise RuntimeError(f"axon_start_nrt_profile rc={rc}")
        try:
            yield
        finally:
            n = lib.axon_stop_nrt_profile(str(output_dir).encode())
            print(f"profile: {n} file(s) written to {output_dir}", file=sys.stderr)

    return _hook


if "antenv.axon_hooks" not in sys.modules:
    _hook = _ntff_profile_via_ctypes("/opt/axon/libaxon_pjrt.so")
    _mod = types.ModuleType("antenv.axon_hooks")
    _mod.get_axon_ntff_profile_hook = lambda: _hook
    sys.modules["antenv.axon_hooks"] = _mod

# ---------------------------------------------------------------------------

B, C, H, W = 8, 256, 48, 48
N = H * W  # 2304
SCALE = (C // 8) ** (-0.5)
EPS = 1e-6
CT = C // 128  # 2 channel tiles
MT = N // 128  # 18 m (key-token) tiles
CHUNKS = [(0, 512), (512, 512), (1024, 512), (1536, 512), (2048, 256)]

BF16 = ml_dtypes.bfloat16

_cache = {}
last_results = None  # BassKernelResults of the most recent run (for test.py)


def _build_program():
    import concourse.bacc as bacc
    import concourse.tile as tile
    import concourse.mybir as mybir
    from contextlib import ExitStack

    f32 = mybir.dt.float32
    bf16 = mybir.dt.bfloat16
    ADD = mybir.AluOpType.add
    SUB = mybir.AluOpType.subtract

    nc = bacc.Bacc("TRN2", target_bir_lowering=False, debug=False)

    x1_d = nc.dram_tensor("x1", [C, N], f32, kind="ExternalInput").ap()
    x2_d = nc.dram_tensor("x2", [C, N], f32, kind="ExternalInput").ap()
    wqt_d = nc.dram_tensor("wqt", [C, C], bf16, kind="ExternalInput").ap()
    wkt_d = nc.dram_tensor("wkt", [C, C], bf16, kind="ExternalInput").ap()
    wvt_d = nc.dram_tensor("wvt", [C, C], bf16, kind="ExternalInput").ap()
    wpt_d = nc.dram_tensor("wpt", [C, C], bf16, kind="ExternalInput").ap()
    # cvec columns: 0/1 = bq per o-tile, 2/3 = bk per o-tile, 4/5 = bp per
    # c-tile, 6:134 = 1.0 (f32 ones row used as K=1 lhsT for broadcasts).
    cvec_d = nc.dram_tensor("cvec", [128, 134], f32, kind="ExternalInput").ap()
    # cbf columns: 0:128 = 1/C (stats broadcast matmul), 128 = 1.0 (rowsum
    # lhsT), 132:260 = 1.0 (K=1 ones lhsT row on partition 0).
    cbf_d = nc.dram_tensor("cbf", [128, 260], bf16, kind="ExternalInput").ap()
    bvrow_d = nc.dram_tensor("bvrow", [1, C], bf16, kind="ExternalInput").ap()
    out_d = nc.dram_tensor("out", [C, N], f32, kind="ExternalOutput").ap()

    with tile.TileContext(nc) as tc, ExitStack() as ctx:
        persist = ctx.enter_context(tc.tile_pool(name="persist", bufs=1))

        # ---- constants -------------------------------------------------
        cvec = persist.tile([128, 134], f32, tag="cvec", name="cvec")
        nc.sync.dma_start(cvec[:], cvec_d[:, :])
        cbf = persist.tile([128, 260], bf16, tag="cbf", name="cbf")
        nc.sync.dma_start(cbf[:], cbf_d[:, :])
        bvrow = persist.tile([1, C], bf16, tag="bvrow", name="bvrow")
        nc.sync.dma_start(bvrow[:], bvrow_d[:, :])

        w_tiles = {}
        for nm, d in (("k", wkt_d), ("v", wvt_d), ("q", wqt_d), ("p", wpt_d)):
            for ct in range(CT):
                t = persist.tile([128, C], bf16, tag=f"w{nm}{ct}", name=f"w{nm}{ct}")
                nc.sync.dma_start(t[:], d[ct * 128 : (ct + 1) * 128, :])
                w_tiles[(nm, ct)] = t

        # persistent intermediates
        k_t = [persist.tile([128, N], bf16, tag=f"k{ot}", name=f"k{ot}") for ot in range(CT)]
        vT_t = [persist.tile([128, C], bf16, tag=f"vT{m}", name=f"vT{m}") for m in range(MT)]
        ou_t = [persist.tile([128, N], bf16, tag=f"ou{ct}", name=f"ou{ct}") for ct in range(CT)]
        rs_sb = persist.tile([1, N], f32, tag="rs", name="rs")
        inv_b = persist.tile([128, N], f32, tag="invb", name="invb")
        out_t = [persist.tile([128, N], f32, tag=f"out{ct}", name=f"out{ct}") for ct in range(CT)]
        x1_t = []
        for ct in range(CT):
            t = persist.tile([128, N], f32, tag=f"x1_{ct}", name=f"x1_{ct}")
            for off, w in CHUNKS:
                nc.sync.dma_start(
                    t[:, off : off + w], x1_d[ct * 128 : (ct + 1) * 128, off : off + w]
                )
            x1_t.append(t)

        with tc.tile_pool(name="mid1", bufs=1) as mid1:
            xn = {}
            with (
                tc.tile_pool(name="mid2", bufs=1) as mid2,
                tc.tile_pool(name="scr", bufs=2) as scr,
                tc.tile_pool(name="ps_st", bufs=2, space="PSUM") as ps_stats,
                tc.tile_pool(name="ps_pj", bufs=2, space="PSUM") as ps_pj,
            ):
                x2_t = []
                for ct in range(CT):
                    t = mid2.tile([128, N], f32, tag=f"x2_{ct}", name=f"x2_{ct}")
                    for off, w in CHUNKS:
                        nc.sync.dma_start(
                            t[:, off : off + w],
                            x2_d[ct * 128 : (ct + 1) * 128, off : off + w],
                        )
                    x2_t.append(t)

                # x2 pipeline first (k and vT gate all of attention); casts
                # for x2 on GpSimd, x1 on DVE so the two streams overlap.
                xb = {}
                for tsel, srct, eng in ((1, x2_t, nc.gpsimd), (0, x1_t, nc.vector)):
                    for ct in range(CT):
                        xb[(tsel, ct)] = mid2.tile(
                            [128, N], bf16, tag=f"xb{tsel}{ct}", name=f"xb{tsel}{ct}"
                        )
                        for off, w in CHUNKS:
                            eng.tensor_copy(
                                xb[(tsel, ct)][:, off : off + w],
                                srct[ct][:, off : off + w],
                            )

                # ---- stats + xn per (tensor, chunk) --------------------
                for tsel in (1, 0):
                    for ji, (off, w) in enumerate(CHUNKS):
                        ub = ps_stats.tile([128, w], f32, tag="ub", name="ub")
                        ms = ps_stats.tile([128, w], f32, tag="ms", name="ms")
                        for ct in range(CT):
                            nc.tensor.matmul(
                                ub[:],
                                cbf[:, 0:128],
                                xb[(tsel, ct)][:, off : off + w],
                                start=(ct == 0),
                                stop=(ct == CT - 1),
                            )
                        for ct in range(CT):
                            xsq_c = scr.tile([128, w], bf16, tag="xsqc", name="xsqc")
                            nc.gpsimd.tensor_mul(
                                xsq_c[:],
                                xb[(tsel, ct)][:, off : off + w],
                                xb[(tsel, ct)][:, off : off + w],
                            )
                            nc.tensor.matmul(
                                ms[:],
                                cbf[:, 0:128],
                                xsq_c[:],
                                start=(ct == 0),
                                stop=(ct == CT - 1),
                            )
                        usq = scr.tile([128, w], f32, tag="usq", name="usq")
                        nc.scalar.square(usq[:], ub[:])
                        var = scr.tile([128, w], f32, tag="var", name="var")
                        # var = (ms + eps) - u^2  (eps folded as an immediate)
                        nc.vector.scalar_tensor_tensor(var[:], ms[:], EPS, usq[:], ADD, SUB)
                        std = scr.tile([128, w], f32, tag="std", name="std")
                        nc.scalar.activation(std[:], var[:], mybir.ActivationFunctionType.Sqrt)
                        rstd = scr.tile([128, w], f32, tag="rstd", name="rstd")
                        nc.vector.reciprocal_approx_fast(rstd[:], std[:])
                        pool = mid2 if tsel == 1 else mid1
                        for ct in range(CT):
                            d = scr.tile([128, w], f32, tag="xnd", name="xnd")
                            nc.vector.tensor_sub(d[:], xb[(tsel, ct)][:, off : off + w], ub[:])
                            xt = pool.tile([128, w], bf16, tag=f"xn{tsel}{ct}{ji}", name=f"xn{tsel}{ct}{ji}")
                            nc.vector.tensor_mul(xt[:], d[:], rstd[:])
                            xn[(tsel, ct, ji)] = xt

                # ---- k projection -------------------------------------
                for ot in range(CT):
                    for ji, (off, w) in enumerate(CHUNKS):
                        ps = ps_pj.tile([128, 512], f32, tag="pj", name="pj")
                        for ct in range(CT):
                            nc.tensor.matmul(
                                ps[:, :w],
                                w_tiles[("k", ct)][:, ot * 128 : (ot + 1) * 128],
                                xn[(1, ct, ji)][:],
                                start=(ct == 0),
                                stop=(ct == CT - 1),
                            )
                        nc.vector.tensor_scalar_add(
                            k_t[ot][:, off : off + w], ps[:, :w], cvec[:, 2 + ot : 3 + ot]
                        )

                # ---- vT (v produced directly transposed) ---------------
                for m in range(MT):
                    col = m * 128
                    ji = min(col // 512, len(CHUNKS) - 1)
                    coff = col - CHUNKS[ji][0]
                    ps = ps_pj.tile([128, C], f32, tag="pv", name="pv")
                    for ct in range(CT):
                        nc.tensor.matmul(
                            ps[:],
                            xn[(1, ct, ji)][:, coff : coff + 128],
                            w_tiles[("v", ct)][:, :],
                            start=(ct == 0),
                            stop=False,
                        )
                    nc.tensor.matmul(
                        ps[:], cbf[0:1, 132:260], bvrow[0:1, :], start=False, stop=True
                    )
                    nc.vector.tensor_copy(vT_t[m][:], ps[:])

            # ---- attention: q projected per chunk, pipelined one ahead -
            with (
                tc.tile_pool(name="qch", bufs=2) as qch,
                tc.tile_pool(name="pt", bufs=2) as pt_pool,
                tc.tile_pool(name="ps_qp", bufs=2, space="PSUM") as ps_qp,
                tc.tile_pool(name="ps_qk", bufs=2, space="PSUM") as ps_qk,
                tc.tile_pool(name="ps_o", bufs=2, space="PSUM") as ps_o,
                tc.tile_pool(name="ps_rs", bufs=2, space="PSUM") as ps_rs,
            ):
                q_ch = {}

                def emit_qproj(ji):
                    off, w = CHUNKS[ji]
                    for ot in range(CT):
                        ps = ps_qp.tile([128, 512], f32, tag="qp", name="qp")
                        for ct in range(CT):
                            nc.tensor.matmul(
                                ps[:, :w],
                                w_tiles[("q", ct)][:, ot * 128 : (ot + 1) * 128],
                                xn[(0, ct, ji)][:],
                                start=(ct == 0),
                                stop=(ct == CT - 1),
                            )
                        qt = qch.tile([128, 512], bf16, tag=f"q{ot}", name=f"q{ot}")
                        nc.vector.tensor_scalar_add(
                            qt[:, :w], ps[:, :w], cvec[:, 0 + ot : 1 + ot]
                        )
                        q_ch[(ji, ot)] = qt

                emit_qproj(0)
                for ji, (off, w) in enumerate(CHUNKS):
                    if ji + 1 < len(CHUNKS):
                        emit_qproj(ji + 1)
                    st = {}

                    def emit_qk(m):
                        ps = ps_qk.tile([128, 512], f32, tag="st", name="st")
                        for ot in range(CT):
                            nc.tensor.matmul(
                                ps[:, :w],
                                k_t[ot][:, m * 128 : (m + 1) * 128],
                                q_ch[(ji, ot)][:, :w],
                                start=(ot == 0),
                                stop=(ot == CT - 1),
                            )
                        st[m] = ps

                    o_ps = [ps_o.tile([128, 512], f32, tag="o", name="o") for _ in range(CT)]
                    rs_ps = ps_rs.tile([1, 512], f32, tag="rsp", name="rsp")

                    emit_qk(0)
                    for m in range(MT):
                        if m + 1 < MT:
                            emit_qk(m + 1)
                        pt = pt_pool.tile([128, w], bf16, tag=f"pt{m}", name=f"pt{m}")
                        nc.scalar.activation(
                            pt[:], st[m][:, :w], mybir.ActivationFunctionType.Exp
                        )
                        del st[m]
                        for c in range(CT):
                            nc.tensor.matmul(
                                o_ps[c][:, :w],
                                vT_t[m][:, c * 128 : (c + 1) * 128],
                                pt[:],
                                start=(m == 0),
                                stop=(m == MT - 1),
                            )
                        nc.tensor.matmul(
                            rs_ps[:, :w],
                            cbf[:, 128:129],
                            pt[:],
                            start=(m == 0),
                            stop=(m == MT - 1),
                        )
                    for c in range(CT):
                        nc.vector.tensor_copy(ou_t[c][:, off : off + w], o_ps[c][:, :w])
                    nc.vector.tensor_copy(rs_sb[0:1, off : off + w], rs_ps[0:1, :w])

        # ---- 1/rowsum broadcast, Wp projection, residual ---------------
        with (
            tc.tile_pool(name="fscr", bufs=4) as fscr,
            tc.tile_pool(name="ps_bc", bufs=2, space="PSUM") as ps_bc,
            tc.tile_pool(name="ps_p", bufs=4, space="PSUM") as ps_p,
        ):
            for ji, (off, w) in enumerate(CHUNKS):
                bc = ps_bc.tile([128, 512], f32, tag="bc", name="bc")
                nc.tensor.matmul(
                    bc[:, :w], cvec[0:1, 6:134], rs_sb[0:1, off : off + w],
                    start=True, stop=True,
                )
                nc.vector.reciprocal_approx_fast(inv_b[:, off : off + w], bc[:, :w])

            for ct in range(CT):
                for ji, (off, w) in enumerate(CHUNKS):
                    ps = ps_p.tile([128, 512], f32, tag="pp", name="pp")
                    for ci in range(CT):
                        nc.tensor.matmul(
                            ps[:, :w],
                            w_tiles[("p", ci)][:, ct * 128 : (ct + 1) * 128],
                            ou_t[ci][:, off : off + w],
                            start=(ci == 0),
                            stop=(ci == CT - 1),
                        )
                    sc = fscr.tile([128, 512], f32, tag="fs", name="fs")
                    nc.vector.tensor_mul(sc[:, :w], ps[:, :w], inv_b[:, off : off + w])
                    nc.vector.scalar_tensor_tensor(
                        out_t[ct][:, off : off + w],
                        sc[:, :w],
                        cvec[:, 4 + ct : 5 + ct],
                        x1_t[ct][:, off : off + w],
                        ADD,
                        ADD,
                    )
                nc.sync.dma_start(out_d[ct * 128 : (ct + 1) * 128, :], out_t[ct][:])

    nc.compile()
    return nc


def _host_prep(inputs):
    f = lambda k: np.asarray(inputs[k], dtype=np.float32)
    Wq, Wk, Wv, Wp = f("Wq"), f("Wk"), f("Wv"), f("Wp")
    bq, bk, bv, bp = f("bq"), f("bk"), f("bv"), f("bp")
    w_nq, b_nq, w_nkv, b_nkv = f("w_nq"), f("b_nq"), f("w_nkv"), f("b_nkv")

    Wq_eff = Wq * w_nq[None, :] * SCALE
    bq_eff = SCALE * (bq + Wq @ b_nq)
    Wk_eff = Wk * w_nkv[None, :]
    bk_eff = bk + Wk @ b_nkv
    Wv_eff = Wv * w_nkv[None, :]
    bv_eff = bv + Wv @ b_nkv

    wqt = np.ascontiguousarray(Wq_eff.T).astype(BF16)
    wkt = np.ascontiguousarray(Wk_eff.T).astype(BF16)
    wvt = np.ascontiguousarray(Wv_eff.T).astype(BF16)
    wpt = np.ascontiguousarray(Wp.T).astype(BF16)

    cvec = np.zeros((128, 134), np.float32)
    cvec[:, 0] = bq_eff[0:128]
    cvec[:, 1] = bq_eff[128:256]
    cvec[:, 2] = bk_eff[0:128]
    cvec[:, 3] = bk_eff[128:256]
    cvec[:, 4] = bp[0:128]
    cvec[:, 5] = bp[128:256]
    cvec[:, 6:134] = 1.0

    cbf = np.zeros((128, 260), np.float32)
    cbf[:, 0:128] = 1.0 / C
    cbf[:, 128] = 1.0
    cbf[:, 132:260] = 1.0
    cbf = cbf.astype(BF16)

    bvrow = bv_eff.reshape(1, C).astype(BF16)
    return dict(wqt=wqt, wkt=wkt, wvt=wvt, wpt=wpt, cvec=cvec, cbf=cbf, bvrow=bvrow)


def _maybe_patch_ldw_opt():
    if os.environ.get("BASS_LDW_OPT", "0") != "1":
        return
    import concourse.bass_utils as bu
    if getattr(bu, "_ldw_patch", False):
        return
    orig = bu.run_command
    def patched(argv, **kw):
        if isinstance(argv, list):
            argv = [a.replace("--enable-ldw-opt=false", "--enable-ldw-opt=true") for a in argv]
        return orig(argv, **kw)
    bu.run_command = patched
    bu._ldw_patch = True


def kernel(**inputs):
    global last_results
    _maybe_patch_ldw_opt()
    from concourse.bass_utils import run_bass_kernel_spmd

    if "nc" not in _cache:
        _cache["nc"] = _build_program()
    nc = _cache["nc"]

    shared = _host_prep(inputs)
    x1 = np.asarray(inputs["x1"], dtype=np.float32).reshape(B, C, N)
    x2 = np.asarray(inputs["x2"], dtype=np.float32).reshape(B, C, N)

    in_maps = []
    for b in range(B):
        m = dict(shared)
        m["x1"] = np.ascontiguousarray(x1[b])
        m["x2"] = np.ascontiguousarray(x2[b])
        in_maps.append(m)

    trace = os.environ.get("BASS_KERNEL_TRACE", "0") == "1"
    res = run_bass_kernel_spmd(
        nc, in_maps, core_ids=list(range(B)), trace=trace
    )
    last_results = res
    out = np.stack([res.results[b]["out"].reshape(C, H, W) for b in range(B)])
    return out.astype(np.float32)



# revision 6
# speedup vs baseline: 1.2019x; 1.2019x over previous
"""Trainium2 Bass kernel for nn_CrossAttentionBlock (B=8, C=256, H=W=48).

Sharding: data-parallel over batch B — one batch per NeuronCore (8 cores).

Per-core math (x: [C=256, N=2304] f32):
  LayerNorm folded into projection weights on host (W_eff = W * w_n,
  b_eff = b + W @ b_n, attention SCALE folded into Wq).
  Stats (mean / mean-square) matmuls run in fp32r DIRECTLY on the raw
  f32 DMA'd activations (fp32r streams 1 cycle/row like bf16 for free
  dims >= 256) — no bf16 cast pass at all. x**2 for the mean-square is
  produced bf16 on Scalar (x2) / GpSimd (x1), both idle early.
  xn = (x - u) * rstd on DVE straight from f32, output bf16.
  Attention transposed: St[m,n] = sum_o k[o,m] q[o,n]; P = exp(St)
  (logits bounded, no row-max). Softmax denominator: P tiles are
  accumulated elementwise on GpSimd (idle during attention) into
  P_sum, then ONE ones-matmul per chunk broadcasts the column sum to
  all 128 partitions (replaces 18 M=1 rowsum matmuls per chunk).
  1/rowsum folded in AFTER the output projection (commutes with Wp).
  Output projection + residual + DMA-out are fused per chunk so the
  tail fully overlaps attention compute.
"""

import os
import sys
import types
import ctypes
import contextlib

sys.path.insert(0, "/opt/trn_rl_repo")

import numpy as np
import ml_dtypes

# ---------------------------------------------------------------------------
# NTFF profile hook stub (antenv.axon_hooks is absent in this container; the
# ctypes shim mirrors trn_agent_boot). Only used when tracing is requested.
# ---------------------------------------------------------------------------


def _ntff_profile_via_ctypes(so_path):
    try:
        lib = ctypes.CDLL(so_path)
    except OSError:
        return None
    if not hasattr(lib, "axon_start_nrt_profile"):
        return None
    lib.axon_start_nrt_profile.argtypes = [
        ctypes.POINTER(ctypes.c_int64),
        ctypes.c_size_t,
    ]
    lib.axon_start_nrt_profile.restype = ctypes.c_int64
    lib.axon_stop_nrt_profile.argtypes = [ctypes.c_char_p]
    lib.axon_stop_nrt_profile.restype = ctypes.c_int64

    @contextlib.contextmanager
    def _hook(output_dir, device_ids):
        import jax

        jax.devices()
        if device_ids:
            ids = (ctypes.c_int64 * len(device_ids))(*device_ids)
            rc = lib.axon_start_nrt_profile(ids, len(device_ids))
        else:
            rc = lib.axon_start_nrt_profile(None, 0)
        if rc != 0:
            raise RuntimeError(f"axon_start_nrt_profile rc={rc}")
        try:
            yield
        finally:
            n = lib.axon_stop_nrt_profile(str(output_dir).encode())
            print(f"profile: {n} file(s) written to {output_dir}", file=sys.stderr)

    return _hook


if "antenv.axon_hooks" not in sys.modules:
    _hook = _ntff_profile_via_ctypes("/opt/axon/libaxon_pjrt.so")
    _mod = types.ModuleType("antenv.axon_hooks")
    _mod.get_axon_ntff_profile_hook = lambda: _hook
    sys.modules["antenv.axon_hooks"] = _mod

# ---------------------------------------------------------------------------

B, C, H, W = 8, 256, 48, 48
N = H * W  # 2304
SCALE = (C // 8) ** (-0.5)
EPS = 1e-6
CT = C // 128  # 2 channel tiles
MT = N // 128  # 18 m (key-token) tiles
CHUNKS = [(0, 512), (512, 512), (1024, 512), (1536, 512), (2048, 256)]

BF16 = ml_dtypes.bfloat16

_cache = {}
last_results = None  # BassKernelResults of the most recent run (for test.py)


def _build_program():
    import concourse.bacc as bacc
    import concourse.tile as tile
    import concourse.mybir as mybir
    from contextlib import ExitStack

    f32 = mybir.dt.float32
    f32r = mybir.dt.float32r
    bf16 = mybir.dt.bfloat16
    ADD = mybir.AluOpType.add
    SUB = mybir.AluOpType.subtract

    nc = bacc.Bacc("TRN2", target_bir_lowering=False, debug=False)

    x1_d = nc.dram_tensor("x1", [C, N], f32r, kind="ExternalInput").ap()
    x2_d = nc.dram_tensor("x2", [C, N], f32r, kind="ExternalInput").ap()
    wqt_d = nc.dram_tensor("wqt", [C, C], bf16, kind="ExternalInput").ap()
    wkt_d = nc.dram_tensor("wkt", [C, C], bf16, kind="ExternalInput").ap()
    wvt_d = nc.dram_tensor("wvt", [C, C], bf16, kind="ExternalInput").ap()
    wpt_d = nc.dram_tensor("wpt", [C, C], bf16, kind="ExternalInput").ap()
    # cvec columns: 0/1 = bq per o-tile, 2/3 = bk per o-tile, 4/5 = bp per
    # c-tile, 6:134 = 1.0 (f32 ones block, lhsT of the colsum-broadcast).
    cvec_d = nc.dram_tensor("cvec", [128, 134], f32, kind="ExternalInput").ap()
    # onesr: fp32r 1/C block, lhsT of the mean matmul on raw f32 x.
    onesr_d = nc.dram_tensor("onesr", [128, 128], f32r, kind="ExternalInput").ap()
    # cbf columns: 0:128 = 1/C (mean-square matmul lhsT), 132:260 = 1.0
    # (K=1 ones lhsT row on partition 0, for the vT bias matmul).
    cbf_d = nc.dram_tensor("cbf", [128, 260], bf16, kind="ExternalInput").ap()
    bvrow_d = nc.dram_tensor("bvrow", [1, C], bf16, kind="ExternalInput").ap()
    out_d = nc.dram_tensor("out", [C, N], f32, kind="ExternalOutput").ap()

    with tile.TileContext(nc) as tc, ExitStack() as ctx:
        persist = ctx.enter_context(tc.tile_pool(name="persist", bufs=1))

        # ---- constants -------------------------------------------------
        cvec = persist.tile([128, 134], f32, tag="cvec", name="cvec")
        nc.sync.dma_start(cvec[:], cvec_d[:, :])
        onesr = persist.tile([128, 128], f32r, tag="onesr", name="onesr")
        nc.sync.dma_start(onesr[:], onesr_d[:, :])
        cbf = persist.tile([128, 260], bf16, tag="cbf", name="cbf")
        nc.sync.dma_start(cbf[:], cbf_d[:, :])
        bvrow = persist.tile([1, C], bf16, tag="bvrow", name="bvrow")
        nc.sync.dma_start(bvrow[:], bvrow_d[:, :])

        w_tiles = {}
        for nm, d in (("k", wkt_d), ("v", wvt_d), ("q", wqt_d), ("p", wpt_d)):
            for ct in range(CT):
                t = persist.tile([128, C], bf16, tag=f"w{nm}{ct}", name=f"w{nm}{ct}")
                nc.sync.dma_start(t[:], d[ct * 128 : (ct + 1) * 128, :])
                w_tiles[(nm, ct)] = t

        # ---- input DMA: x2 first (k/vT gate attention), chunk-major ----
        x2_t = [
            persist.tile([128, N], f32r, tag=f"x2_{ct}", name=f"x2_{ct}")
            for ct in range(CT)
        ]
        for off, w in CHUNKS:
            for ct in range(CT):
                nc.sync.dma_start(
                    x2_t[ct][:, off : off + w],
                    x2_d[ct * 128 : (ct + 1) * 128, off : off + w],
                )
        x1_t = [
            persist.tile([128, N], f32r, tag=f"x1_{ct}", name=f"x1_{ct}")
            for ct in range(CT)
        ]
        for off, w in CHUNKS:
            for ct in range(CT):
                nc.sync.dma_start(
                    x1_t[ct][:, off : off + w],
                    x1_d[ct * 128 : (ct + 1) * 128, off : off + w],
                )
        x1_f = [t[:].bitcast(f32) for t in x1_t]
        x2_f = [t[:].bitcast(f32) for t in x2_t]

        # persistent intermediates
        k_t = [persist.tile([128, N], bf16, tag=f"k{ot}", name=f"k{ot}") for ot in range(CT)]
        vT_t = [persist.tile([128, C], bf16, tag=f"vT{m}", name=f"vT{m}") for m in range(MT)]
        xn1 = {}  # (ct, ji) -> bf16 [128, w], persists into attention
        xn1_t = [
            persist.tile([128, N], bf16, tag=f"xn1_{ct}", name=f"xn1_{ct}")
            for ct in range(CT)
        ]

        with (
            tc.tile_pool(name="mid2", bufs=1) as mid2,
            tc.tile_pool(name="scr", bufs=3) as scr,
            tc.tile_pool(name="ps_st", bufs=3, space="PSUM") as ps_stats,
            tc.tile_pool(name="ps_pj", bufs=2, space="PSUM") as ps_pj,
        ):
            # ---- x2: stats + xn, chunk-pipelined ----------------------
            xn2 = {}
            for ji, (off, w) in enumerate(CHUNKS):
                ub = ps_stats.tile([128, 512], f32, tag="st2", name="ub")
                for ct in range(CT):
                    nc.tensor.matmul(
                        ub[:, :w],
                        onesr[:, :],
                        x2_t[ct][:, off : off + w],
                        start=(ct == 0),
                        stop=(ct == CT - 1),
                    )
                ms = ps_stats.tile([128, 512], f32, tag="st2", name="ms")
                for ct in range(CT):
                    xsq = scr.tile([128, 512], bf16, tag="xsqc", name="xsqc")
                    nc.scalar.square(xsq[:, :w], x2_f[ct][:, off : off + w])
                    nc.tensor.matmul(
                        ms[:, :w],
                        cbf[:, 0:128],
                        xsq[:, :w],
                        start=(ct == 0),
                        stop=(ct == CT - 1),
                    )
                usq = scr.tile([128, 512], f32, tag="usq", name="usq")
                nc.scalar.square(usq[:, :w], ub[:, :w])
                var = scr.tile([128, 512], f32, tag="var", name="var")
                nc.vector.scalar_tensor_tensor(
                    var[:, :w], ms[:, :w], EPS, usq[:, :w], ADD, SUB
                )
                std = scr.tile([128, 512], f32, tag="std", name="std")
                nc.scalar.activation(
                    std[:, :w], var[:, :w], mybir.ActivationFunctionType.Sqrt
                )
                rstd = scr.tile([128, 512], f32, tag="rstd", name="rstd")
                nc.vector.reciprocal_approx_fast(rstd[:, :w], std[:, :w])
                for ct in range(CT):
                    d = scr.tile([128, 512], f32, tag="xnd", name="xnd")
                    nc.vector.tensor_sub(
                        d[:, :w], x2_f[ct][:, off : off + w], ub[:, :w]
                    )
                    xt = mid2.tile([128, 512], bf16, tag=f"xn2_{ct}{ji}", name=f"xn2_{ct}{ji}")
                    nc.vector.tensor_mul(xt[:, :w], d[:, :w], rstd[:, :w])
                    xn2[(ct, ji)] = xt

                # ---- k projection for this chunk ----------------------
                for ot in range(CT):
                    ps = ps_pj.tile([128, 512], f32, tag="pj", name="pj")
                    for ct in range(CT):
                        nc.tensor.matmul(
                            ps[:, :w],
                            w_tiles[("k", ct)][:, ot * 128 : (ot + 1) * 128],
                            xn2[(ct, ji)][:, :w],
                            start=(ct == 0),
                            stop=(ct == CT - 1),
                        )
                    nc.vector.tensor_scalar_add(
                        k_t[ot][:, off : off + w], ps[:, :w], cvec[:, 2 + ot : 3 + ot]
                    )

                # ---- vT for the m-tiles inside this chunk --------------
                for m in range(off // 128, (off + w) // 128):
                    coff = m * 128 - off
                    ps = ps_pj.tile([128, C], f32, tag="pv", name="pv")
                    for ct in range(CT):
                        nc.tensor.matmul(
                            ps[:],
                            xn2[(ct, ji)][:, coff : coff + 128],
                            w_tiles[("v", ct)][:, :],
                            start=(ct == 0),
                            stop=False,
                        )
                    nc.tensor.matmul(
                        ps[:], cbf[0:1, 132:260], bvrow[0:1, :], start=False, stop=True
                    )
                    nc.vector.tensor_copy(vT_t[m][:], ps[:])

        # ---- attention + fused epilogue ---------------------------------
        with (
            tc.tile_pool(name="qch", bufs=2) as qch,
            tc.tile_pool(name="pt", bufs=3) as pt_pool,
            tc.tile_pool(name="ascr", bufs=3) as ascr,
            tc.tile_pool(name="ps_qp", bufs=1, space="PSUM") as ps_qp,
            tc.tile_pool(name="ps_st1", bufs=1, space="PSUM") as ps_st1,
            tc.tile_pool(name="ps_qk", bufs=2, space="PSUM") as ps_qk,
            tc.tile_pool(name="ps_o", bufs=2, space="PSUM") as ps_o,
            tc.tile_pool(name="ps_bc", bufs=1, space="PSUM") as ps_bc,
            tc.tile_pool(name="ps_p", bufs=1, space="PSUM") as ps_p,
        ):
            q_ch = {}

            def emit_x1_chunk(ji):
                # stats1 (squares on GpSimd) + xn1 + q projection for chunk ji
                off, w = CHUNKS[ji]
                ub = ps_st1.tile([128, 512], f32, tag="st1", name="ub1")
                for ct in range(CT):
                    nc.tensor.matmul(
                        ub[:, :w],
                        onesr[:, :],
                        x1_t[ct][:, off : off + w],
                        start=(ct == 0),
                        stop=(ct == CT - 1),
                    )
                ms = ps_st1.tile([128, 512], f32, tag="st1", name="ms1")
                for ct in range(CT):
                    xsq = ascr.tile([128, 512], bf16, tag="xsq1", name="xsq1")
                    nc.gpsimd.tensor_mul(
                        xsq[:, :w],
                        x1_f[ct][:, off : off + w],
                        x1_f[ct][:, off : off + w],
                    )
                    nc.tensor.matmul(
                        ms[:, :w],
                        cbf[:, 0:128],
                        xsq[:, :w],
                        start=(ct == 0),
                        stop=(ct == CT - 1),
                    )
                usq = ascr.tile([128, 512], f32, tag="usq1", name="usq1")
                nc.scalar.square(usq[:, :w], ub[:, :w])
                var = ascr.tile([128, 512], f32, tag="var1", name="var1")
                nc.vector.scalar_tensor_tensor(
                    var[:, :w], ms[:, :w], EPS, usq[:, :w], ADD, SUB
                )
                std = ascr.tile([128, 512], f32, tag="std1", name="std1")
                nc.scalar.activation(
                    std[:, :w], var[:, :w], mybir.ActivationFunctionType.Sqrt
                )
                rstd = ascr.tile([128, 512], f32, tag="rstd1", name="rstd1")
                nc.vector.reciprocal_approx_fast(rstd[:, :w], std[:, :w])
                for ct in range(CT):
                    d = ascr.tile([128, 512], f32, tag="xnd1", name="xnd1")
                    nc.vector.tensor_sub(
                        d[:, :w], x1_f[ct][:, off : off + w], ub[:, :w]
                    )
                    nc.vector.tensor_mul(
                        xn1_t[ct][:, off : off + w], d[:, :w], rstd[:, :w]
                    )
                # q projection
                for ot in range(CT):
                    ps = ps_qp.tile([128, 512], f32, tag="qp", name="qp")
                    for ct in range(CT):
                        nc.tensor.matmul(
                            ps[:, :w],
                            w_tiles[("q", ct)][:, ot * 128 : (ot + 1) * 128],
                            xn1_t[ct][:, off : off + w],
                            start=(ct == 0),
                            stop=(ct == CT - 1),
                        )
                    qt = qch.tile([128, 512], bf16, tag=f"q{ot}", name=f"q{ot}")
                    nc.vector.tensor_scalar_add(
                        qt[:, :w], ps[:, :w], cvec[:, 0 + ot : 1 + ot]
                    )
                    q_ch[(ji, ot)] = qt

            emit_x1_chunk(0)
            for ji, (off, w) in enumerate(CHUNKS):
                st = {}

                def emit_qk(m):
                    ps = ps_qk.tile([128, 512], f32, tag="st", name="st")
                    for ot in range(CT):
                        nc.tensor.matmul(
                            ps[:, :w],
                            k_t[ot][:, m * 128 : (m + 1) * 128],
                            q_ch[(ji, ot)][:, :w],
                            start=(ot == 0),
                            stop=(ot == CT - 1),
                        )
                    st[m] = ps

                o_ps = [ps_o.tile([128, 512], f32, tag="o", name="o") for _ in range(CT)]
                psum_acc = ascr.tile([128, 512], f32, tag="psum", name="psum")
                pts = {}

                emit_qk(0)
                for m in range(MT):
                    if m + 1 < MT:
                        emit_qk(m + 1)
                    if m == 8 and ji + 1 < len(CHUNKS):
                        emit_x1_chunk(ji + 1)
                    pt = pt_pool.tile([128, 512], bf16, tag=f"pt{m%3}", name=f"pt{m%3}")
                    nc.scalar.activation(
                        pt[:, :w], st[m][:, :w], mybir.ActivationFunctionType.Exp
                    )
                    del st[m]
                    for c in range(CT):
                        nc.tensor.matmul(
                            o_ps[c][:, :w],
                            vT_t[m][:, c * 128 : (c + 1) * 128],
                            pt[:, :w],
                            start=(m == 0),
                            stop=(m == MT - 1),
                        )
                    # accumulate softmax denominator on GpSimd (idle here)
                    if m == 0:
                        pts[0] = pt
                    elif m == 1:
                        nc.gpsimd.tensor_add(
                            psum_acc[:, :w], pts[0][:, :w], pt[:, :w]
                        )
                        del pts[0]
                    else:
                        nc.gpsimd.tensor_add(
                            psum_acc[:, :w], psum_acc[:, :w], pt[:, :w]
                        )

                # colsum broadcast to all partitions in one f32 matmul
                bc = ps_bc.tile([128, 512], f32, tag="bc", name="bc")
                nc.tensor.matmul(
                    bc[:, :w], cvec[:, 6:134], psum_acc[:, :w], start=True, stop=True
                )
                inv_b = ascr.tile([128, 512], f32, tag="invb", name="invb")
                nc.vector.reciprocal_approx_fast(inv_b[:, :w], bc[:, :w])

                ou = []
                for c in range(CT):
                    t = ascr.tile([128, 512], bf16, tag=f"ou{c}", name=f"ou{c}")
                    nc.vector.tensor_copy(t[:, :w], o_ps[c][:, :w])
                    ou.append(t)

                # ---- fused output projection + residual + DMA-out ------
                for ct in range(CT):
                    ps = ps_p.tile([128, 512], f32, tag="pp", name="pp")
                    for ci in range(CT):
                        nc.tensor.matmul(
                            ps[:, :w],
                            w_tiles[("p", ci)][:, ct * 128 : (ct + 1) * 128],
                            ou[ci][:, :w],
                            start=(ci == 0),
                            stop=(ci == CT - 1),
                        )
                    sc = ascr.tile([128, 512], f32, tag="fs", name="fs")
                    nc.vector.tensor_mul(sc[:, :w], ps[:, :w], inv_b[:, :w])
                    ot_t = ascr.tile([128, 512], f32, tag=f"out{ct}", name=f"out{ct}")
                    nc.vector.scalar_tensor_tensor(
                        ot_t[:, :w],
                        sc[:, :w],
                        cvec[:, 4 + ct : 5 + ct],
                        x1_f[ct][:, off : off + w],
                        ADD,
                        ADD,
                    )
                    nc.sync.dma_start(
                        out_d[ct * 128 : (ct + 1) * 128, off : off + w], ot_t[:, :w]
                    )

    nc.compile()
    return nc


def _host_prep(inputs):
    f = lambda k: np.asarray(inputs[k], dtype=np.float32)
    Wq, Wk, Wv, Wp = f("Wq"), f("Wk"), f("Wv"), f("Wp")
    bq, bk, bv, bp = f("bq"), f("bk"), f("bv"), f("bp")
    w_nq, b_nq, w_nkv, b_nkv = f("w_nq"), f("b_nq"), f("w_nkv"), f("b_nkv")

    Wq_eff = Wq * w_nq[None, :] * SCALE
    bq_eff = SCALE * (bq + Wq @ b_nq)
    Wk_eff = Wk * w_nkv[None, :]
    bk_eff = bk + Wk @ b_nkv
    Wv_eff = Wv * w_nkv[None, :]
    bv_eff = bv + Wv @ b_nkv

    wqt = np.ascontiguousarray(Wq_eff.T).astype(BF16)
    wkt = np.ascontiguousarray(Wk_eff.T).astype(BF16)
    wvt = np.ascontiguousarray(Wv_eff.T).astype(BF16)
    wpt = np.ascontiguousarray(Wp.T).astype(BF16)

    cvec = np.zeros((128, 134), np.float32)
    cvec[:, 0] = bq_eff[0:128]
    cvec[:, 1] = bq_eff[128:256]
    cvec[:, 2] = bk_eff[0:128]
    cvec[:, 3] = bk_eff[128:256]
    cvec[:, 4] = bp[0:128]
    cvec[:, 5] = bp[128:256]
    cvec[:, 6:134] = 1.0

    onesr = np.full((128, 128), 1.0 / C, np.float32)

    cbf = np.zeros((128, 260), np.float32)
    cbf[:, 0:128] = 1.0 / C
    cbf[:, 128] = 1.0
    cbf[:, 132:260] = 1.0
    cbf = cbf.astype(BF16)

    bvrow = bv_eff.reshape(1, C).astype(BF16)
    return dict(
        wqt=wqt, wkt=wkt, wvt=wvt, wpt=wpt, cvec=cvec, onesr=onesr, cbf=cbf,
        bvrow=bvrow,
    )


def _maybe_patch_ldw_opt():
    if os.environ.get("BASS_LDW_OPT", "0") != "1":
        return
    import concourse.bass_utils as bu
    if getattr(bu, "_ldw_patch", False):
        return
    orig = bu.run_command
    def patched(argv, **kw):
        if isinstance(argv, list):
            argv = [a.replace("--enable-ldw-opt=false", "--enable-ldw-opt=true") for a in argv]
        return orig(argv, **kw)
    bu.run_command = patched
    bu._ldw_patch = True


def kernel(**inputs):
    global last_results
    _maybe_patch_ldw_opt()
    from concourse.bass_utils import run_bass_kernel_spmd

    if "nc" not in _cache:
        _cache["nc"] = _build_program()
    nc = _cache["nc"]

    shared = _host_prep(inputs)
    x1 = np.asarray(inputs["x1"], dtype=np.float32).reshape(B, C, N)
    x2 = np.asarray(inputs["x2"], dtype=np.float32).reshape(B, C, N)

    in_maps = []
    for b in range(B):
        m = dict(shared)
        m["x1"] = np.ascontiguousarray(x1[b])
        m["x2"] = np.ascontiguousarray(x2[b])
        in_maps.append(m)

    trace = os.environ.get("BASS_KERNEL_TRACE", "0") == "1"
    res = run_bass_kernel_spmd(
        nc, in_maps, core_ids=list(range(B)), trace=trace
    )
    last_results = res
    out = np.stack([res.results[b]["out"].reshape(C, H, W) for b in range(B)])
    return out.astype(np.float32)


# revision 7
# speedup vs baseline: 1.3655x; 1.1361x over previous
"""Trainium2 Bass kernel for nn_CrossAttentionBlock (B=8, C=256, H=W=48).

Sharding: data-parallel over batch B — one batch per NeuronCore (8 cores).

Per-core math (x: [C=256, N=2304] f32):
  LayerNorm folded into projection weights on host (W_eff = W * w_n,
  b_eff = b + W @ b_n, attention SCALE folded into Wq).
  Stats (mean / mean-square) matmuls run in fp32r DIRECTLY on the raw
  f32 DMA'd activations (fp32r streams 1 cycle/row like bf16 for free
  dims >= 256) — no bf16 cast pass at all. x**2 for the mean-square is
  produced bf16 on Scalar (x2) / GpSimd (x1), both idle early.
  xn = (x - u) * rstd on DVE straight from f32, output bf16.
  Attention transposed: St[m,n] = sum_o k[o,m] q[o,n]; P = exp(St)
  (logits bounded, no row-max). Softmax denominator: P tiles are
  accumulated elementwise on GpSimd (idle during attention) into
  P_sum, then ONE ones-matmul per chunk broadcasts the column sum to
  all 128 partitions (replaces 18 M=1 rowsum matmuls per chunk).
  1/rowsum folded in AFTER the output projection (commutes with Wp).
  Output projection + residual + DMA-out are fused per chunk so the
  tail fully overlaps attention compute.
"""

import os
import sys
import types
import ctypes
import contextlib

sys.path.insert(0, "/opt/trn_rl_repo")

import numpy as np
import ml_dtypes

# ---------------------------------------------------------------------------
# NTFF profile hook stub (antenv.axon_hooks is absent in this container; the
# ctypes shim mirrors trn_agent_boot). Only used when tracing is requested.
# ---------------------------------------------------------------------------


def _ntff_profile_via_ctypes(so_path):
    try:
        lib = ctypes.CDLL(so_path)
    except OSError:
        return None
    if not hasattr(lib, "axon_start_nrt_profile"):
        return None
    lib.axon_start_nrt_profile.argtypes = [
        ctypes.POINTER(ctypes.c_int64),
        ctypes.c_size_t,
    ]
    lib.axon_start_nrt_profile.restype = ctypes.c_int64
    lib.axon_stop_nrt_profile.argtypes = [ctypes.c_char_p]
    lib.axon_stop_nrt_profile.restype = ctypes.c_int64

    @contextlib.contextmanager
    def _hook(output_dir, device_ids):
        import jax

        jax.devices()
        if device_ids:
            ids = (ctypes.c_int64 * len(device_ids))(*device_ids)
            rc = lib.axon_start_nrt_profile(ids, len(device_ids))
        else:
            rc = lib.axon_start_nrt_profile(None, 0)
        if rc != 0:
            raise RuntimeError(f"axon_start_nrt_profile rc={rc}")
        try:
            yield
        finally:
            n = lib.axon_stop_nrt_profile(str(output_dir).encode())
            print(f"profile: {n} file(s) written to {output_dir}", file=sys.stderr)

    return _hook


if "antenv.axon_hooks" not in sys.modules:
    _hook = _ntff_profile_via_ctypes("/opt/axon/libaxon_pjrt.so")
    _mod = types.ModuleType("antenv.axon_hooks")
    _mod.get_axon_ntff_profile_hook = lambda: _hook
    sys.modules["antenv.axon_hooks"] = _mod

# ---------------------------------------------------------------------------

B, C, H, W = 8, 256, 48, 48
N = H * W  # 2304
SCALE = (C // 8) ** (-0.5)
EPS = 1e-6
CT = C // 128  # 2 channel tiles
MT = N // 128  # 18 m (key-token) tiles
CHUNKS = [(0, 512), (512, 512), (1024, 512), (1536, 512), (2048, 256)]

BF16 = ml_dtypes.bfloat16

_cache = {}
last_results = None  # BassKernelResults of the most recent run (for test.py)


def _build_program():
    import concourse.bacc as bacc
    import concourse.tile as tile
    import concourse.mybir as mybir
    from contextlib import ExitStack

    f32 = mybir.dt.float32
    f32r = mybir.dt.float32r
    bf16 = mybir.dt.bfloat16
    ADD = mybir.AluOpType.add
    SUB = mybir.AluOpType.subtract

    nc = bacc.Bacc("TRN2", target_bir_lowering=False, debug=False)

    x1_d = nc.dram_tensor("x1", [C, N], f32r, kind="ExternalInput").ap()
    x2_d = nc.dram_tensor("x2", [C, N], f32r, kind="ExternalInput").ap()
    wqt_d = nc.dram_tensor("wqt", [C, C], bf16, kind="ExternalInput").ap()
    wkt_d = nc.dram_tensor("wkt", [C, C], bf16, kind="ExternalInput").ap()
    wvt_d = nc.dram_tensor("wvt", [C, C], bf16, kind="ExternalInput").ap()
    wpt_d = nc.dram_tensor("wpt", [C, C], bf16, kind="ExternalInput").ap()
    # cvec columns: 0/1 = bq per o-tile, 2/3 = bk per o-tile, 4/5 = bp per
    # c-tile, 6:134 = 1.0 (f32 ones block, lhsT of the colsum-broadcast).
    cvec_d = nc.dram_tensor("cvec", [128, 134], f32, kind="ExternalInput").ap()
    # onesr: fp32r 1/C block, lhsT of the mean matmul on raw f32 x.
    onesr_d = nc.dram_tensor("onesr", [128, 128], f32r, kind="ExternalInput").ap()
    # cbf columns: 0:128 = 1/C (mean-square matmul lhsT), 132:260 = 1.0
    # (K=1 ones lhsT row on partition 0, for the vT bias matmul).
    cbf_d = nc.dram_tensor("cbf", [128, 260], bf16, kind="ExternalInput").ap()
    bvrow_d = nc.dram_tensor("bvrow", [1, C], bf16, kind="ExternalInput").ap()
    out_d = nc.dram_tensor("out", [C, N], f32, kind="ExternalOutput").ap()

    with tile.TileContext(nc) as tc, ExitStack() as ctx:
        persist = ctx.enter_context(tc.tile_pool(name="persist", bufs=1))

        # ---- constants -------------------------------------------------
        cvec = persist.tile([128, 134], f32, tag="cvec", name="cvec")
        nc.sync.dma_start(cvec[:], cvec_d[:, :])
        onesr = persist.tile([128, 128], f32r, tag="onesr", name="onesr")
        nc.sync.dma_start(onesr[:], onesr_d[:, :])
        cbf = persist.tile([128, 260], bf16, tag="cbf", name="cbf")
        nc.sync.dma_start(cbf[:], cbf_d[:, :])
        bvrow = persist.tile([1, C], bf16, tag="bvrow", name="bvrow")
        nc.sync.dma_start(bvrow[:], bvrow_d[:, :])

        w_tiles = {}
        for nm, d in (("k", wkt_d), ("v", wvt_d), ("q", wqt_d), ("p", wpt_d)):
            for ct in range(CT):
                t = persist.tile([128, C], bf16, tag=f"w{nm}{ct}", name=f"w{nm}{ct}")
                nc.sync.dma_start(t[:], d[ct * 128 : (ct + 1) * 128, :])
                w_tiles[(nm, ct)] = t

        # ---- input DMA: x2 first (k/vT gate attention), chunk-major ----
        x2_t = [
            persist.tile([128, N], f32r, tag=f"x2_{ct}", name=f"x2_{ct}")
            for ct in range(CT)
        ]
        for off, w in CHUNKS:
            for ct in range(CT):
                nc.sync.dma_start(
                    x2_t[ct][:, off : off + w],
                    x2_d[ct * 128 : (ct + 1) * 128, off : off + w],
                )
        x1_t = [
            persist.tile([128, N], f32r, tag=f"x1_{ct}", name=f"x1_{ct}")
            for ct in range(CT)
        ]
        for off, w in CHUNKS:
            for ct in range(CT):
                nc.sync.dma_start(
                    x1_t[ct][:, off : off + w],
                    x1_d[ct * 128 : (ct + 1) * 128, off : off + w],
                )
        x1_f = [t[:].bitcast(f32) for t in x1_t]
        x2_f = [t[:].bitcast(f32) for t in x2_t]

        # persistent intermediates
        k_t = [persist.tile([128, N], bf16, tag=f"k{ot}", name=f"k{ot}") for ot in range(CT)]
        vT_t = [persist.tile([128, C], bf16, tag=f"vT{m}", name=f"vT{m}") for m in range(MT)]
        xn1 = {}  # (ct, ji) -> bf16 [128, w], persists into attention
        xn1_t = [
            persist.tile([128, N], bf16, tag=f"xn1_{ct}", name=f"xn1_{ct}")
            for ct in range(CT)
        ]

        with (
            tc.tile_pool(name="mid2", bufs=1) as mid2,
            tc.tile_pool(name="scr", bufs=3) as scr,
            tc.tile_pool(name="ps_st", bufs=3, space="PSUM") as ps_stats,
            tc.tile_pool(name="ps_pj", bufs=2, space="PSUM") as ps_pj,
        ):
            # ---- x2: stats + xn, chunk-pipelined ----------------------
            xn2 = {}
            for ji, (off, w) in enumerate(CHUNKS):
                ub = ps_stats.tile([128, 512], f32, tag="st2", name="ub")
                for ct in range(CT):
                    nc.tensor.matmul(
                        ub[:, :w],
                        onesr[:, :],
                        x2_t[ct][:, off : off + w],
                        start=(ct == 0),
                        stop=(ct == CT - 1),
                    )
                ms = ps_stats.tile([128, 512], f32, tag="st2", name="ms")
                for ct in range(CT):
                    xsq = scr.tile([128, 512], bf16, tag="xsqc", name="xsqc")
                    nc.scalar.square(xsq[:, :w], x2_f[ct][:, off : off + w])
                    nc.tensor.matmul(
                        ms[:, :w],
                        cbf[:, 0:128],
                        xsq[:, :w],
                        start=(ct == 0),
                        stop=(ct == CT - 1),
                    )
                usq = scr.tile([128, 512], f32, tag="usq", name="usq")
                nc.scalar.square(usq[:, :w], ub[:, :w])
                var = scr.tile([128, 512], f32, tag="var", name="var")
                nc.vector.scalar_tensor_tensor(
                    var[:, :w], ms[:, :w], EPS, usq[:, :w], ADD, SUB
                )
                std = scr.tile([128, 512], f32, tag="std", name="std")
                nc.scalar.activation(
                    std[:, :w], var[:, :w], mybir.ActivationFunctionType.Sqrt
                )
                rstd = scr.tile([128, 512], f32, tag="rstd", name="rstd")
                nc.vector.reciprocal_approx_fast(rstd[:, :w], std[:, :w])
                for ct in range(CT):
                    d = scr.tile([128, 512], f32, tag="xnd", name="xnd")
                    nc.vector.tensor_sub(
                        d[:, :w], x2_f[ct][:, off : off + w], ub[:, :w]
                    )
                    xt = mid2.tile([128, 512], bf16, tag=f"xn2_{ct}{ji}", name=f"xn2_{ct}{ji}")
                    eng = nc.gpsimd if ct == 0 else nc.vector
                    eng.tensor_mul(xt[:, :w], d[:, :w], rstd[:, :w])
                    xn2[(ct, ji)] = xt

                # ---- k projection for this chunk ----------------------
                for ot in range(CT):
                    ps = ps_pj.tile([128, 512], f32, tag="pj", name="pj")
                    for ct in range(CT):
                        nc.tensor.matmul(
                            ps[:, :w],
                            w_tiles[("k", ct)][:, ot * 128 : (ot + 1) * 128],
                            xn2[(ct, ji)][:, :w],
                            start=(ct == 0),
                            stop=(ct == CT - 1),
                        )
                    nc.vector.tensor_scalar_add(
                        k_t[ot][:, off : off + w], ps[:, :w], cvec[:, 2 + ot : 3 + ot]
                    )

                # ---- vT for the m-tiles inside this chunk --------------
                for m in range(off // 128, (off + w) // 128):
                    coff = m * 128 - off
                    ps = ps_pj.tile([128, C], f32, tag="pv", name="pv")
                    for ct in range(CT):
                        nc.tensor.matmul(
                            ps[:],
                            xn2[(ct, ji)][:, coff : coff + 128],
                            w_tiles[("v", ct)][:, :],
                            start=(ct == 0),
                            stop=False,
                        )
                    nc.tensor.matmul(
                        ps[:], cbf[0:1, 132:260], bvrow[0:1, :], start=False, stop=True
                    )
                    nc.vector.tensor_copy(vT_t[m][:], ps[:])

        # ---- attention + fused epilogue ---------------------------------
        with (
            tc.tile_pool(name="qch", bufs=2) as qch,
            tc.tile_pool(name="pt", bufs=3) as pt_pool,
            tc.tile_pool(name="ascr", bufs=3) as ascr,
            tc.tile_pool(name="ps_qp", bufs=1, space="PSUM") as ps_qp,
            tc.tile_pool(name="ps_st1", bufs=1, space="PSUM") as ps_st1,
            tc.tile_pool(name="ps_qk", bufs=2, space="PSUM") as ps_qk,
            tc.tile_pool(name="ps_o", bufs=2, space="PSUM") as ps_o,
            tc.tile_pool(name="ps_bc", bufs=1, space="PSUM") as ps_bc,
            tc.tile_pool(name="ps_p", bufs=1, space="PSUM") as ps_p,
        ):
            q_ch = {}

            def emit_x1_chunk(ji):
                # stats1 (squares on GpSimd) + xn1 + q projection for chunk ji
                off, w = CHUNKS[ji]
                ub = ps_st1.tile([128, 512], f32, tag="st1", name="ub1")
                for ct in range(CT):
                    nc.tensor.matmul(
                        ub[:, :w],
                        onesr[:, :],
                        x1_t[ct][:, off : off + w],
                        start=(ct == 0),
                        stop=(ct == CT - 1),
                    )
                ms = ps_st1.tile([128, 512], f32, tag="st1", name="ms1")
                for ct in range(CT):
                    xsq = ascr.tile([128, 512], bf16, tag="xsq1", name="xsq1")
                    nc.gpsimd.tensor_mul(
                        xsq[:, :w],
                        x1_f[ct][:, off : off + w],
                        x1_f[ct][:, off : off + w],
                    )
                    nc.tensor.matmul(
                        ms[:, :w],
                        cbf[:, 0:128],
                        xsq[:, :w],
                        start=(ct == 0),
                        stop=(ct == CT - 1),
                    )
                usq = ascr.tile([128, 512], f32, tag="usq1", name="usq1")
                nc.scalar.square(usq[:, :w], ub[:, :w])
                var = ascr.tile([128, 512], f32, tag="var1", name="var1")
                nc.vector.scalar_tensor_tensor(
                    var[:, :w], ms[:, :w], EPS, usq[:, :w], ADD, SUB
                )
                std = ascr.tile([128, 512], f32, tag="std1", name="std1")
                nc.scalar.activation(
                    std[:, :w], var[:, :w], mybir.ActivationFunctionType.Sqrt
                )
                rstd = ascr.tile([128, 512], f32, tag="rstd1", name="rstd1")
                nc.vector.reciprocal_approx_fast(rstd[:, :w], std[:, :w])
                for ct in range(CT):
                    d = ascr.tile([128, 512], f32, tag="xnd1", name="xnd1")
                    nc.vector.tensor_sub(
                        d[:, :w], x1_f[ct][:, off : off + w], ub[:, :w]
                    )
                    eng = nc.gpsimd if ct == 0 else nc.vector
                    eng.tensor_mul(
                        xn1_t[ct][:, off : off + w], d[:, :w], rstd[:, :w]
                    )
                # q projection
                for ot in range(CT):
                    ps = ps_qp.tile([128, 512], f32, tag="qp", name="qp")
                    for ct in range(CT):
                        nc.tensor.matmul(
                            ps[:, :w],
                            w_tiles[("q", ct)][:, ot * 128 : (ot + 1) * 128],
                            xn1_t[ct][:, off : off + w],
                            start=(ct == 0),
                            stop=(ct == CT - 1),
                        )
                    qt = qch.tile([128, 512], bf16, tag=f"q{ot}", name=f"q{ot}")
                    nc.vector.tensor_scalar_add(
                        qt[:, :w], ps[:, :w], cvec[:, 0 + ot : 1 + ot]
                    )
                    q_ch[(ji, ot)] = qt

            emit_x1_chunk(0)
            for ji, (off, w) in enumerate(CHUNKS):
                st = {}

                def emit_qk(m):
                    ps = ps_qk.tile([128, 512], f32, tag="st", name="st")
                    for ot in range(CT):
                        nc.tensor.matmul(
                            ps[:, :w],
                            k_t[ot][:, m * 128 : (m + 1) * 128],
                            q_ch[(ji, ot)][:, :w],
                            start=(ot == 0),
                            stop=(ot == CT - 1),
                        )
                    st[m] = ps

                o_ps = [ps_o.tile([128, 512], f32, tag="o", name="o") for _ in range(CT)]
                psum_acc = ascr.tile([128, 512], bf16, tag="psum", name="psum")
                pts = {}

                emit_qk(0)
                for m in range(MT):
                    if m + 1 < MT:
                        emit_qk(m + 1)
                    if m == 8 and ji + 1 < len(CHUNKS):
                        emit_x1_chunk(ji + 1)
                    pt = pt_pool.tile([128, 512], bf16, tag=f"pt{m%3}", name=f"pt{m%3}")
                    nc.scalar.activation(
                        pt[:, :w], st[m][:, :w], mybir.ActivationFunctionType.Exp
                    )
                    del st[m]
                    for c in range(CT):
                        nc.tensor.matmul(
                            o_ps[c][:, :w],
                            vT_t[m][:, c * 128 : (c + 1) * 128],
                            pt[:, :w],
                            start=(m == 0),
                            stop=(m == MT - 1),
                        )
                    # accumulate softmax denominator on GpSimd (idle here)
                    if m == 0:
                        pts[0] = pt
                    elif m == 1:
                        nc.vector.tensor_add(
                            psum_acc[:, :w], pts[0][:, :w], pt[:, :w]
                        )
                        del pts[0]
                    else:
                        nc.vector.tensor_add(
                            psum_acc[:, :w], psum_acc[:, :w], pt[:, :w]
                        )

                # colsum broadcast to all partitions in one f32 matmul
                bc = ps_bc.tile([128, 512], f32, tag="bc", name="bc")
                nc.tensor.matmul(
                    bc[:, :w], cbf[:, 132:260], psum_acc[:, :w], start=True, stop=True
                )
                inv_b = ascr.tile([128, 512], f32, tag="invb", name="invb")
                nc.vector.reciprocal_approx_fast(inv_b[:, :w], bc[:, :w])

                ou = []
                for c in range(CT):
                    t = ascr.tile([128, 512], bf16, tag=f"ou{c}", name=f"ou{c}")
                    nc.vector.tensor_copy(t[:, :w], o_ps[c][:, :w])
                    ou.append(t)

                # ---- fused output projection + residual + DMA-out ------
                for ct in range(CT):
                    ps = ps_p.tile([128, 512], f32, tag="pp", name="pp")
                    for ci in range(CT):
                        nc.tensor.matmul(
                            ps[:, :w],
                            w_tiles[("p", ci)][:, ct * 128 : (ct + 1) * 128],
                            ou[ci][:, :w],
                            start=(ci == 0),
                            stop=(ci == CT - 1),
                        )
                    sc = ascr.tile([128, 512], f32, tag="fs", name="fs")
                    nc.vector.tensor_mul(sc[:, :w], ps[:, :w], inv_b[:, :w])
                    ot_t = ascr.tile([128, 512], f32, tag=f"out{ct}", name=f"out{ct}")
                    nc.vector.scalar_tensor_tensor(
                        ot_t[:, :w],
                        sc[:, :w],
                        cvec[:, 4 + ct : 5 + ct],
                        x1_f[ct][:, off : off + w],
                        ADD,
                        ADD,
                    )
                    nc.sync.dma_start(
                        out_d[ct * 128 : (ct + 1) * 128, off : off + w], ot_t[:, :w]
                    )

    nc.compile()
    return nc


def _host_prep(inputs):
    f = lambda k: np.asarray(inputs[k], dtype=np.float32)
    Wq, Wk, Wv, Wp = f("Wq"), f("Wk"), f("Wv"), f("Wp")
    bq, bk, bv, bp = f("bq"), f("bk"), f("bv"), f("bp")
    w_nq, b_nq, w_nkv, b_nkv = f("w_nq"), f("b_nq"), f("w_nkv"), f("b_nkv")

    Wq_eff = Wq * w_nq[None, :] * SCALE
    bq_eff = SCALE * (bq + Wq @ b_nq)
    Wk_eff = Wk * w_nkv[None, :]
    bk_eff = bk + Wk @ b_nkv
    Wv_eff = Wv * w_nkv[None, :]
    bv_eff = bv + Wv @ b_nkv

    wqt = np.ascontiguousarray(Wq_eff.T).astype(BF16)
    wkt = np.ascontiguousarray(Wk_eff.T).astype(BF16)
    wvt = np.ascontiguousarray(Wv_eff.T).astype(BF16)
    wpt = np.ascontiguousarray(Wp.T).astype(BF16)

    cvec = np.zeros((128, 134), np.float32)
    cvec[:, 0] = bq_eff[0:128]
    cvec[:, 1] = bq_eff[128:256]
    cvec[:, 2] = bk_eff[0:128]
    cvec[:, 3] = bk_eff[128:256]
    cvec[:, 4] = bp[0:128]
    cvec[:, 5] = bp[128:256]
    cvec[:, 6:134] = 1.0

    onesr = np.full((128, 128), 1.0 / C, np.float32)

    cbf = np.zeros((128, 260), np.float32)
    cbf[:, 0:128] = 1.0 / C
    cbf[:, 128] = 1.0
    cbf[:, 132:260] = 1.0
    cbf = cbf.astype(BF16)

    bvrow = bv_eff.reshape(1, C).astype(BF16)
    return dict(
        wqt=wqt, wkt=wkt, wvt=wvt, wpt=wpt, cvec=cvec, onesr=onesr, cbf=cbf,
        bvrow=bvrow,
    )


def _maybe_patch_ldw_opt():
    if os.environ.get("BASS_LDW_OPT", "0") != "1":
        return
    import concourse.bass_utils as bu
    if getattr(bu, "_ldw_patch", False):
        return
    orig = bu.run_command
    def patched(argv, **kw):
        if isinstance(argv, list):
            argv = [a.replace("--enable-ldw-opt=false", "--enable-ldw-opt=true") for a in argv]
        return orig(argv, **kw)
    bu.run_command = patched
    bu._ldw_patch = True


def kernel(**inputs):
    global last_results
    _maybe_patch_ldw_opt()
    from concourse.bass_utils import run_bass_kernel_spmd

    if "nc" not in _cache:
        _cache["nc"] = _build_program()
    nc = _cache["nc"]

    shared = _host_prep(inputs)
    x1 = np.asarray(inputs["x1"], dtype=np.float32).reshape(B, C, N)
    x2 = np.asarray(inputs["x2"], dtype=np.float32).reshape(B, C, N)

    in_maps = []
    for b in range(B):
        m = dict(shared)
        m["x1"] = np.ascontiguousarray(x1[b])
        m["x2"] = np.ascontiguousarray(x2[b])
        in_maps.append(m)

    trace = os.environ.get("BASS_KERNEL_TRACE", "0") == "1"
    res = run_bass_kernel_spmd(
        nc, in_maps, core_ids=list(range(B)), trace=trace
    )
    last_results = res
    out = np.stack([res.results[b]["out"].reshape(C, H, W) for b in range(B)])
    return out.astype(np.float32)


# revision 8
# speedup vs baseline: 1.4402x; 1.0547x over previous
"""Trainium2 Bass kernel for nn_CrossAttentionBlock (B=8, C=256, H=W=48).

Sharding: data-parallel over batch B — one batch per NeuronCore (8 cores).

Per-core math (x: [C=256, N=2304] f32):
  LayerNorm folded into projection weights on host (W_eff = W * w_n,
  b_eff = b + W @ b_n, attention SCALE folded into Wq). The k-bias is
  dropped entirely (a per-query-column logit shift cancels in softmax);
  the v-bias is folded into the output-projection bias (bv contributes
  bv (x) rowsum to the unnormalized output, which normalizes to a
  constant): bp_eff = bp + Wp @ bv_eff.
  Stats (mean / mean-square) matmuls run in fp32r DIRECTLY on the raw
  f32 DMA'd activations (fp32r streams 1 cycle/row like bf16 for free
  dims >= 256) — no bf16 cast pass at all. x**2 for the mean-square is
  produced bf16 on Scalar (x2) / GpSimd (x1), both idle early.
  xn = (x - u) * rstd on DVE/GpSimd straight from f32, output bf16.
  Attention transposed: St[m,n] = sum_o k[o,m] q[o,n]; P = exp(St)
  (logits bounded, no row-max). Softmax denominator: P tiles are
  accumulated elementwise on Vector (bf16) into P_sum, then ONE
  ones-matmul per chunk broadcasts the column sum to all partitions
  (replaces 18 M=1 rowsum matmuls per chunk). 1/rowsum folded in
  AFTER the output projection (commutes with Wp).
  Software pipelining: query-chunk 0's attention m-tiles are merged
  into the x2 stats/k/vT production loop (each x2 chunk yields 4 more
  k/vT m-tiles, immediately consumed by chunk-0 S/exp/PV), so
  attention starts as soon as the first x2 chunk lands. Output
  projection + residual + DMA-out are fused per chunk.
"""

import os
import sys
import types
import ctypes
import contextlib

sys.path.insert(0, "/opt/trn_rl_repo")

import numpy as np
import ml_dtypes

# ---------------------------------------------------------------------------
# NTFF profile hook stub (antenv.axon_hooks is absent in this container; the
# ctypes shim mirrors trn_agent_boot). Only used when tracing is requested.
# ---------------------------------------------------------------------------


def _ntff_profile_via_ctypes(so_path):
    try:
        lib = ctypes.CDLL(so_path)
    except OSError:
        return None
    if not hasattr(lib, "axon_start_nrt_profile"):
        return None
    lib.axon_start_nrt_profile.argtypes = [
        ctypes.POINTER(ctypes.c_int64),
        ctypes.c_size_t,
    ]
    lib.axon_start_nrt_profile.restype = ctypes.c_int64
    lib.axon_stop_nrt_profile.argtypes = [ctypes.c_char_p]
    lib.axon_stop_nrt_profile.restype = ctypes.c_int64

    @contextlib.contextmanager
    def _hook(output_dir, device_ids):
        import jax

        jax.devices()
        if device_ids:
            ids = (ctypes.c_int64 * len(device_ids))(*device_ids)
            rc = lib.axon_start_nrt_profile(ids, len(device_ids))
        else:
            rc = lib.axon_start_nrt_profile(None, 0)
        if rc != 0:
            raise RuntimeError(f"axon_start_nrt_profile rc={rc}")
        try:
            yield
        finally:
            n = lib.axon_stop_nrt_profile(str(output_dir).encode())
            print(f"profile: {n} file(s) written to {output_dir}", file=sys.stderr)

    return _hook


if "antenv.axon_hooks" not in sys.modules:
    _hook = _ntff_profile_via_ctypes("/opt/axon/libaxon_pjrt.so")
    _mod = types.ModuleType("antenv.axon_hooks")
    _mod.get_axon_ntff_profile_hook = lambda: _hook
    sys.modules["antenv.axon_hooks"] = _mod

# ---------------------------------------------------------------------------

B, C, H, W = 8, 256, 48, 48
N = H * W  # 2304
SCALE = (C // 8) ** (-0.5)
EPS = 1e-6
CT = C // 128  # 2 channel tiles
MT = N // 128  # 18 m (key-token) tiles
CHUNKS = [(0, 512), (512, 512), (1024, 512), (1536, 512), (2048, 256)]

BF16 = ml_dtypes.bfloat16

_cache = {}
last_results = None  # BassKernelResults of the most recent run (for test.py)


def _build_program():
    import concourse.bacc as bacc
    import concourse.tile as tile
    import concourse.mybir as mybir
    from contextlib import ExitStack

    f32 = mybir.dt.float32
    f32r = mybir.dt.float32r
    bf16 = mybir.dt.bfloat16
    ADD = mybir.AluOpType.add
    SUB = mybir.AluOpType.subtract

    nc = bacc.Bacc("TRN2", target_bir_lowering=False, debug=False)

    x1_d = nc.dram_tensor("x1", [C, N], f32r, kind="ExternalInput").ap()
    x2_d = nc.dram_tensor("x2", [C, N], f32r, kind="ExternalInput").ap()
    wqt_d = nc.dram_tensor("wqt", [C, C], bf16, kind="ExternalInput").ap()
    wkt_d = nc.dram_tensor("wkt", [C, C], bf16, kind="ExternalInput").ap()
    wvt_d = nc.dram_tensor("wvt", [C, C], bf16, kind="ExternalInput").ap()
    wpt_d = nc.dram_tensor("wpt", [C, C], bf16, kind="ExternalInput").ap()
    # cvec columns: 0/1 = bq per o-tile, 4/5 = bp_eff per c-tile.
    cvec_d = nc.dram_tensor("cvec", [128, 6], f32, kind="ExternalInput").ap()
    # onesr: fp32r 1/C block, lhsT of the mean matmul on raw f32 x.
    onesr_d = nc.dram_tensor("onesr", [128, 128], f32r, kind="ExternalInput").ap()
    # cbf columns: 0:128 = 1/C (mean-square matmul lhsT), 132:260 = 1.0
    # (ones block, lhsT of the denominator colsum-broadcast matmul).
    cbf_d = nc.dram_tensor("cbf", [128, 260], bf16, kind="ExternalInput").ap()
    out_d = nc.dram_tensor("out", [C, N], f32, kind="ExternalOutput").ap()

    with tile.TileContext(nc) as tc, ExitStack() as ctx:
        persist = ctx.enter_context(tc.tile_pool(name="persist", bufs=1))

        # ---- constants -------------------------------------------------
        cvec = persist.tile([128, 6], f32, tag="cvec", name="cvec")
        nc.sync.dma_start(cvec[:], cvec_d[:, :])
        onesr = persist.tile([128, 128], f32r, tag="onesr", name="onesr")
        nc.sync.dma_start(onesr[:], onesr_d[:, :])
        cbf = persist.tile([128, 260], bf16, tag="cbf", name="cbf")
        nc.sync.dma_start(cbf[:], cbf_d[:, :])

        w_tiles = {}
        for nm, d in (("k", wkt_d), ("v", wvt_d), ("q", wqt_d), ("p", wpt_d)):
            for ct in range(CT):
                t = persist.tile([128, C], bf16, tag=f"w{nm}{ct}", name=f"w{nm}{ct}")
                nc.sync.dma_start(t[:], d[ct * 128 : (ct + 1) * 128, :])
                w_tiles[(nm, ct)] = t

        # ---- input DMA: chunk-major, x2/x1 interleaved -----------------
        x2_t = [
            persist.tile([128, N], f32r, tag=f"x2_{ct}", name=f"x2_{ct}")
            for ct in range(CT)
        ]
        x1_t = [
            persist.tile([128, N], f32r, tag=f"x1_{ct}", name=f"x1_{ct}")
            for ct in range(CT)
        ]
        for off, w in CHUNKS:
            for ct in range(CT):
                nc.sync.dma_start(
                    x2_t[ct][:, off : off + w],
                    x2_d[ct * 128 : (ct + 1) * 128, off : off + w],
                )
            for ct in range(CT):
                nc.sync.dma_start(
                    x1_t[ct][:, off : off + w],
                    x1_d[ct * 128 : (ct + 1) * 128, off : off + w],
                )
        x1_f = [t[:].bitcast(f32) for t in x1_t]
        x2_f = [t[:].bitcast(f32) for t in x2_t]

        # persistent intermediates
        k_t = [persist.tile([128, N], bf16, tag=f"k{ot}", name=f"k{ot}") for ot in range(CT)]
        vT_t = [persist.tile([128, C], bf16, tag=f"vT{m}", name=f"vT{m}") for m in range(MT)]
        xn1_t = [
            persist.tile([128, N], bf16, tag=f"xn1_{ct}", name=f"xn1_{ct}")
            for ct in range(CT)
        ]

        with (
            tc.tile_pool(name="mid2", bufs=1) as mid2,
            tc.tile_pool(name="scr", bufs=3) as scr,
            tc.tile_pool(name="qch", bufs=2) as qch,
            tc.tile_pool(name="pt", bufs=3) as pt_pool,
            tc.tile_pool(name="ascr", bufs=3) as ascr,
            tc.tile_pool(name="ps_a", bufs=1, space="PSUM") as ps_a,   # stats
            tc.tile_pool(name="ps_b", bufs=1, space="PSUM") as ps_b,   # k/q proj
            tc.tile_pool(name="ps_c", bufs=1, space="PSUM") as ps_c,   # vT proj
            tc.tile_pool(name="ps_d", bufs=1, space="PSUM") as ps_d,   # bc + outproj
            tc.tile_pool(name="ps_qk", bufs=2, space="PSUM") as ps_qk,
            tc.tile_pool(name="ps_o", bufs=2, space="PSUM") as ps_o,
        ):
            q_ch = {}

            def emit_x1_chunk(ji):
                # stats1 (squares on GpSimd) + xn1 + q projection for chunk ji
                off, w = CHUNKS[ji]
                ub = ps_a.tile([128, 512], f32, tag="sta", name="ub1")
                for ct in range(CT):
                    nc.tensor.matmul(
                        ub[:, :w],
                        onesr[:, :],
                        x1_t[ct][:, off : off + w],
                        start=(ct == 0),
                        stop=(ct == CT - 1),
                    )
                ms = ps_a.tile([128, 512], f32, tag="sta", name="ms1")
                for ct in range(CT):
                    xsq = ascr.tile([128, 512], bf16, tag="xsq1", name="xsq1")
                    nc.gpsimd.tensor_mul(
                        xsq[:, :w],
                        x1_f[ct][:, off : off + w],
                        x1_f[ct][:, off : off + w],
                    )
                    nc.tensor.matmul(
                        ms[:, :w],
                        cbf[:, 0:128],
                        xsq[:, :w],
                        start=(ct == 0),
                        stop=(ct == CT - 1),
                    )
                usq = ascr.tile([128, 512], f32, tag="usq1", name="usq1")
                nc.scalar.square(usq[:, :w], ub[:, :w])
                var = ascr.tile([128, 512], f32, tag="var1", name="var1")
                nc.vector.scalar_tensor_tensor(
                    var[:, :w], ms[:, :w], EPS, usq[:, :w], ADD, SUB
                )
                std = ascr.tile([128, 512], f32, tag="std1", name="std1")
                nc.scalar.activation(
                    std[:, :w], var[:, :w], mybir.ActivationFunctionType.Sqrt
                )
                rstd = ascr.tile([128, 512], f32, tag="rstd1", name="rstd1")
                nc.vector.reciprocal_approx_fast(rstd[:, :w], std[:, :w])
                for ct in range(CT):
                    d = ascr.tile([128, 512], f32, tag="xnd1", name="xnd1")
                    nc.vector.tensor_sub(
                        d[:, :w], x1_f[ct][:, off : off + w], ub[:, :w]
                    )
                    eng = nc.gpsimd if ct == 0 else nc.vector
                    eng.tensor_mul(
                        xn1_t[ct][:, off : off + w], d[:, :w], rstd[:, :w]
                    )
                # q projection
                for ot in range(CT):
                    ps = ps_b.tile([128, 512], f32, tag="pjq", name="qp")
                    for ct in range(CT):
                        nc.tensor.matmul(
                            ps[:, :w],
                            w_tiles[("q", ct)][:, ot * 128 : (ot + 1) * 128],
                            xn1_t[ct][:, off : off + w],
                            start=(ct == 0),
                            stop=(ct == CT - 1),
                        )
                    qt = qch.tile([128, 512], bf16, tag=f"q{ot}", name=f"q{ot}")
                    nc.vector.tensor_scalar_add(
                        qt[:, :w], ps[:, :w], cvec[:, 0 + ot : 1 + ot]
                    )
                    q_ch[(ji, ot)] = qt

            # ---- per-query-chunk attention state + emitters -------------
            class AttnState:
                pass

            def attn_begin(ji):
                s = AttnState()
                s.ji = ji
                s.off, s.w = CHUNKS[ji]
                s.st = {}
                s.o_ps = [
                    ps_o.tile([128, 512], f32, tag="o", name="o") for _ in range(CT)
                ]
                s.psum_acc = ascr.tile([128, 512], bf16, tag="psum", name="psum")
                s.pt0 = None
                return s

            def attn_qk(s, m):
                ps = ps_qk.tile([128, 512], f32, tag="st", name="st")
                for ot in range(CT):
                    nc.tensor.matmul(
                        ps[:, : s.w],
                        k_t[ot][:, m * 128 : (m + 1) * 128],
                        q_ch[(s.ji, ot)][:, : s.w],
                        start=(ot == 0),
                        stop=(ot == CT - 1),
                    )
                s.st[m] = ps

            def attn_m(s, m):
                # exp + PV + denominator accumulate for m-tile m
                w = s.w
                pt = pt_pool.tile([128, 512], bf16, tag=f"pt{m%3}", name=f"pt{m%3}")
                nc.scalar.activation(
                    pt[:, :w], s.st[m][:, :w], mybir.ActivationFunctionType.Exp
                )
                del s.st[m]
                for c in range(CT):
                    nc.tensor.matmul(
                        s.o_ps[c][:, :w],
                        vT_t[m][:, c * 128 : (c + 1) * 128],
                        pt[:, :w],
                        start=(m == 0),
                        stop=(m == MT - 1),
                    )
                if m == 0:
                    s.pt0 = pt
                elif m == 1:
                    nc.vector.tensor_add(
                        s.psum_acc[:, :w], s.pt0[:, :w], pt[:, :w]
                    )
                    s.pt0 = None
                else:
                    nc.vector.tensor_add(
                        s.psum_acc[:, :w], s.psum_acc[:, :w], pt[:, :w]
                    )

            def attn_end(s):
                # denominator broadcast + output projection + residual + DMA
                w, off = s.w, s.off
                bc = ps_d.tile([128, 512], f32, tag="dd", name="bc")
                nc.tensor.matmul(
                    bc[:, :w], cbf[:, 132:260], s.psum_acc[:, :w],
                    start=True, stop=True,
                )
                inv_b = ascr.tile([128, 512], f32, tag="invb", name="invb")
                nc.vector.reciprocal_approx_fast(inv_b[:, :w], bc[:, :w])
                ou = []
                for c in range(CT):
                    t = ascr.tile([128, 512], bf16, tag=f"ou{c}", name=f"ou{c}")
                    nc.vector.tensor_copy(t[:, :w], s.o_ps[c][:, :w])
                    ou.append(t)
                for ct in range(CT):
                    ps = ps_d.tile([128, 512], f32, tag="dd", name="pp")
                    for ci in range(CT):
                        nc.tensor.matmul(
                            ps[:, :w],
                            w_tiles[("p", ci)][:, ct * 128 : (ct + 1) * 128],
                            ou[ci][:, :w],
                            start=(ci == 0),
                            stop=(ci == CT - 1),
                        )
                    sc = ascr.tile([128, 512], f32, tag="fs", name="fs")
                    nc.vector.tensor_mul(sc[:, :w], ps[:, :w], inv_b[:, :w])
                    ot_t = ascr.tile([128, 512], f32, tag=f"out{ct}", name=f"out{ct}")
                    nc.vector.scalar_tensor_tensor(
                        ot_t[:, :w],
                        sc[:, :w],
                        cvec[:, 4 + ct : 5 + ct],
                        x1_f[ct][:, off : off + w],
                        ADD,
                        ADD,
                    )
                    nc.sync.dma_start(
                        out_d[ct * 128 : (ct + 1) * 128, off : off + w],
                        ot_t[:, :w],
                    )

            # ================= phase 1: x2 pipeline + merged chunk-0 ====
            emit_x1_chunk(0)
            s0 = attn_begin(0)
            for ji, (off, w) in enumerate(CHUNKS):
                # ---- x2 stats for chunk ji ----------------------------
                ub = ps_a.tile([128, 512], f32, tag="sta", name="ub")
                for ct in range(CT):
                    nc.tensor.matmul(
                        ub[:, :w],
                        onesr[:, :],
                        x2_t[ct][:, off : off + w],
                        start=(ct == 0),
                        stop=(ct == CT - 1),
                    )
                ms = ps_a.tile([128, 512], f32, tag="sta", name="ms")
                for ct in range(CT):
                    xsq = scr.tile([128, 512], bf16, tag="xsqc", name="xsqc")
                    nc.scalar.square(xsq[:, :w], x2_f[ct][:, off : off + w])
                    nc.tensor.matmul(
                        ms[:, :w],
                        cbf[:, 0:128],
                        xsq[:, :w],
                        start=(ct == 0),
                        stop=(ct == CT - 1),
                    )
                usq = scr.tile([128, 512], f32, tag="usq", name="usq")
                nc.scalar.square(usq[:, :w], ub[:, :w])
                var = scr.tile([128, 512], f32, tag="var", name="var")
                nc.vector.scalar_tensor_tensor(
                    var[:, :w], ms[:, :w], EPS, usq[:, :w], ADD, SUB
                )
                std = scr.tile([128, 512], f32, tag="std", name="std")
                nc.scalar.activation(
                    std[:, :w], var[:, :w], mybir.ActivationFunctionType.Sqrt
                )
                rstd = scr.tile([128, 512], f32, tag="rstd", name="rstd")
                nc.vector.reciprocal_approx_fast(rstd[:, :w], std[:, :w])
                xn2 = {}
                for ct in range(CT):
                    d = scr.tile([128, 512], f32, tag="xnd", name="xnd")
                    nc.vector.tensor_sub(
                        d[:, :w], x2_f[ct][:, off : off + w], ub[:, :w]
                    )
                    xt = mid2.tile([128, 512], bf16, tag=f"xn2_{ct}{ji}", name=f"xn2_{ct}{ji}")
                    eng = nc.gpsimd if ct == 0 else nc.vector
                    eng.tensor_mul(xt[:, :w], d[:, :w], rstd[:, :w])
                    xn2[ct] = xt

                # ---- k projection for chunk ji (k-bias dropped) --------
                for ot in range(CT):
                    ps = ps_b.tile([128, 512], f32, tag="pjq", name="pj")
                    for ct in range(CT):
                        nc.tensor.matmul(
                            ps[:, :w],
                            w_tiles[("k", ct)][:, ot * 128 : (ot + 1) * 128],
                            xn2[ct][:, :w],
                            start=(ct == 0),
                            stop=(ct == CT - 1),
                        )
                    nc.vector.tensor_copy(k_t[ot][:, off : off + w], ps[:, :w])

                # ---- vT + merged chunk-0 attention m-tiles -------------
                for m in range(off // 128, (off + w) // 128):
                    coff = m * 128 - off
                    ps = ps_c.tile([128, C], f32, tag="pv", name="pv")
                    for ct in range(CT):
                        nc.tensor.matmul(
                            ps[:],
                            xn2[ct][:, coff : coff + 128],
                            w_tiles[("v", ct)][:, :],
                            start=(ct == 0),
                            stop=(ct == CT - 1),
                        )
                    nc.vector.tensor_copy(vT_t[m][:], ps[:])
                    # chunk-0 attention for this m-tile
                    attn_qk(s0, m)
                    attn_m(s0, m)
                if ji == 2:
                    emit_x1_chunk(1)
            attn_end(s0)

            # ================= chunks 1..4: pure attention ===============
            for ji in range(1, len(CHUNKS)):
                s = attn_begin(ji)
                attn_qk(s, 0)
                for m in range(MT):
                    if m + 1 < MT:
                        attn_qk(s, m + 1)
                    if m == 8 and ji + 1 < len(CHUNKS):
                        emit_x1_chunk(ji + 1)
                    attn_m(s, m)
                attn_end(s)

    nc.compile()
    return nc


def _host_prep(inputs):
    f = lambda k: np.asarray(inputs[k], dtype=np.float32)
    Wq, Wk, Wv, Wp = f("Wq"), f("Wk"), f("Wv"), f("Wp")
    bq, bk, bv, bp = f("bq"), f("bk"), f("bv"), f("bp")
    w_nq, b_nq, w_nkv, b_nkv = f("w_nq"), f("b_nq"), f("w_nkv"), f("b_nkv")

    Wq_eff = Wq * w_nq[None, :] * SCALE
    bq_eff = SCALE * (bq + Wq @ b_nq)
    Wk_eff = Wk * w_nkv[None, :]
    Wv_eff = Wv * w_nkv[None, :]
    bv_eff = bv + Wv @ b_nkv
    # v-bias folded into the output-projection bias (see module docstring)
    bp_eff = bp + Wp @ bv_eff

    wqt = np.ascontiguousarray(Wq_eff.T).astype(BF16)
    wkt = np.ascontiguousarray(Wk_eff.T).astype(BF16)
    wvt = np.ascontiguousarray(Wv_eff.T).astype(BF16)
    wpt = np.ascontiguousarray(Wp.T).astype(BF16)

    cvec = np.zeros((128, 6), np.float32)
    cvec[:, 0] = bq_eff[0:128]
    cvec[:, 1] = bq_eff[128:256]
    cvec[:, 4] = bp_eff[0:128]
    cvec[:, 5] = bp_eff[128:256]

    onesr = np.full((128, 128), 1.0 / C, np.float32)

    cbf = np.zeros((128, 260), np.float32)
    cbf[:, 0:128] = 1.0 / C
    cbf[:, 132:260] = 1.0
    cbf = cbf.astype(BF16)

    return dict(wqt=wqt, wkt=wkt, wvt=wvt, wpt=wpt, cvec=cvec, onesr=onesr, cbf=cbf)


def _maybe_patch_ldw_opt():
    if os.environ.get("BASS_LDW_OPT", "0") != "1":
        return
    import concourse.bass_utils as bu
    if getattr(bu, "_ldw_patch", False):
        return
    orig = bu.run_command
    def patched(argv, **kw):
        if isinstance(argv, list):
            argv = [a.replace("--enable-ldw-opt=false", "--enable-ldw-opt=true") for a in argv]
        return orig(argv, **kw)
    bu.run_command = patched
    bu._ldw_patch = True


def kernel(**inputs):
    global last_results
    _maybe_patch_ldw_opt()
    from concourse.bass_utils import run_bass_kernel_spmd

    if "nc" not in _cache:
        _cache["nc"] = _build_program()
    nc = _cache["nc"]

    shared = _host_prep(inputs)
    x1 = np.asarray(inputs["x1"], dtype=np.float32).reshape(B, C, N)
    x2 = np.asarray(inputs["x2"], dtype=np.float32).reshape(B, C, N)

    in_maps = []
    for b in range(B):
        m = dict(shared)
        m["x1"] = np.ascontiguousarray(x1[b])
        m["x2"] = np.ascontiguousarray(x2[b])
        in_maps.append(m)

    trace = os.environ.get("BASS_KERNEL_TRACE", "0") == "1"
    res = run_bass_kernel_spmd(
        nc, in_maps, core_ids=list(range(B)), trace=trace
    )
    last_results = res
    out = np.stack([res.results[b]["out"].reshape(C, H, W) for b in range(B)])
    return out.astype(np.float32)
